# revision 3
# baseline (speedup 1.0000x reference)
"""Multi-head causal self-attention (B=4, T=2048, D=1024, H=16) on 8 TRN2 cores.

Sharding (hardcoded): data-parallel over the 4 batches x tensor-parallel over
head halves. Core c handles batch c//2 and local heads (c%2)*8 .. (c%2)*8+7
for all 2048 positions. Every core runs the same SPMD program on its slice:

  x[b] [2048,1024] --(cast bf16 + on-chip DMA transpose)--> x^T
  Q^T = (Wq_slice)^T x^T / 8,  K^T = (Wk_slice)^T x^T       [dh-pairs packed
  V   = x Wv_slice (+ ones column for the softmax denominator)   on 128 parts]
  S^T = K Q^T per 128x128 block (causal blocks only, PE row-group packing),
  P^T = exp(S^T) (ScalarE, diag blocks masked via a -1e9 triangular matmul),
  ctx = P V  (the 65th column accumulates the softmax denominator l),
  ctx /= l, transpose via PE, partial_out = ctx^T.T @ Wo_slice.

The host sums the two partial outputs per batch and adds the bias bo.
"""
import numpy as np

import concourse.bass as bass
import concourse.mybir as mybir
import concourse.tile as tile
from concourse import bacc
from concourse.bass_utils import run_bass_kernel_spmd
from concourse.masks import make_identity, make_lower_triangular

F32 = mybir.dt.float32
BF16 = mybir.dt.bfloat16
AF = mybir.ActivationFunctionType

B, T, D = 4, 2048, 1024
HL = 8              # local heads per core
HP = HL // 2        # local head pairs (two heads share 128 partitions)
DH = 64
PO = D // 128       # contraction chunks over D
CD = HL * DH        # 512: local context feature dim
FC = CD // 128      # 4
NB = T // 128       # 16 query/key blocks of 128
NTW = T // 512      # 4 transpose/proj column groups
SCALE = 1.0 / 8.0   # 1/sqrt(DH)
NEG = -1e9
CHUNK = 8           # key blocks per S^T psum tile ([128, 1024] = 2 banks)


def _emit_attention_qb(nc, qb, kt_sb, qt_sb, v_sb, ident, utri,
                       ptp, stp, cxp, smallp, ctx_sb):
    """Attention for one query block (all 4 local head pairs)."""
    nkb = qb + 1
    for hp in range(HP):
        pt_e = ptp.tile([128, NB, 128], BF16, tag="pt")
        pt_o = ptp.tile([128, NB, 128], BF16, tag="pt")
        # separate PSUM tiles: a matmul with start=True clears the whole
        # bank's has_written bits, so two accumulation chains cannot
        # interleave within one bank
        ctx_e = cxp.tile([128, 65], F32, tag="cx")
        ctx_o = cxp.tile([128, 65], F32, tag="cx")
        nchunks = (nkb + CHUNK - 1) // CHUNK
        for ch in range(nchunks):
            k0 = ch * CHUNK
            k1 = min(nkb, k0 + CHUNK)
            st_e = stp.tile([128, 128 * CHUNK], F32, tag="st")
            st_o = stp.tile([128, 128 * CHUNK], F32, tag="st")
            for kb in range(k0, k1):
                w = kb - k0
                diag = kb == qb
                for st, lo in ((st_e, 0), (st_o, 64)):
                    nc.tensor.matmul(
                        st[:, w * 128:(w + 1) * 128],
                        lhsT=kt_sb[lo:lo + 64, hp, kb * 128:(kb + 1) * 128],
                        rhs=qt_sb[lo:lo + 64, hp, qb * 128:(qb + 1) * 128],
                        start=True, stop=not diag,
                    )
                    if diag:
                        # add -1e9 to strictly-future (k > q) entries
                        nc.tensor.matmul(
                            st[:, w * 128:(w + 1) * 128],
                            lhsT=ident, rhs=utri, start=False, stop=True,
                        )
            ncol = (k1 - k0) * 128
            nc.scalar.activation(pt_e[:, k0:k1, :], st_e[:, :ncol], AF.Exp)
            nc.scalar.activation(pt_o[:, k0:k1, :], st_o[:, :ncol], AF.Exp)
            for kb in range(k0, k1):
                nc.tensor.matmul(
                    ctx_e, lhsT=pt_e[:, kb, :], rhs=v_sb[:, kb, 2 * hp, :],
                    start=(kb == 0), stop=(kb == nkb - 1),
                )
                nc.tensor.matmul(
                    ctx_o, lhsT=pt_o[:, kb, :], rhs=v_sb[:, kb, 2 * hp + 1, :],
                    start=(kb == 0), stop=(kb == nkb - 1),
                )
        ctmp = smallp.tile([128, 130], F32, tag="ctmp")
        nc.vector.tensor_copy(ctmp[:, 0:65], ctx_e)
        nc.vector.tensor_copy(ctmp[:, 65:130], ctx_o)
        linv = smallp.tile([128, 2], F32, tag="linv")
        nc.vector.reciprocal(linv, ctmp[:, 64::65])
        nc.vector.tensor_scalar_mul(ctx_sb[:, 2 * hp, :], ctmp[:, 0:64], linv[:, 0:1])
        nc.vector.tensor_scalar_mul(ctx_sb[:, 2 * hp + 1, :], ctmp[:, 65:129], linv[:, 1:2])


def _emit_out_proj_qb(nc, qb, ctx_sb, ctxt_sb, wo_sb, ident, mmp, osbp, out_d):
    """ctx^T via PE transpose, then the output projection for one q block."""
    for fc in range(FC):
        tp = mmp.tile([128, 128], BF16, tag="mm")
        nc.tensor.transpose(tp, ctx_sb[:, 2 * fc:2 * fc + 2, :], ident)
        nc.vector.tensor_copy(ctxt_sb[:, fc, :], tp)
    for dw in range(2):
        ps = mmp.tile([128, 512], F32, tag="mm")
        for fc in range(FC):
            nc.tensor.matmul(
                ps, lhsT=ctxt_sb[:, fc, :], rhs=wo_sb[:, fc, dw * 512:(dw + 1) * 512],
                start=(fc == 0), stop=(fc == FC - 1),
            )
        osb = osbp.tile([128, 512], F32, tag="osb")
        nc.vector.tensor_copy(osb, ps)
        nc.scalar.dma_start(out_d[qb * 128:(qb + 1) * 128, dw * 512:(dw + 1) * 512], osb)


def _emit_proj_tw(nc, tw, x_d, xstage, x16p, xtp, mmp,
                  wq_sb, wk_sb, wv_sb, kt_sb, qt_sb, v_sb):
    """Load/cast/transpose 512 columns of x^T; project K^T, Q^T, V for them."""
    xt = xtp.tile([128, PO, 512], BF16, tag="xt")
    for tb in range(4):
        kb = tw * 4 + tb
        xf = xstage.tile([128, D], F32, tag="xf")
        nc.scalar.dma_start(xf, x_d[kb * 128:(kb + 1) * 128, :])
        x16 = x16p.tile([128, D], BF16, tag="x16")
        nc.gpsimd.tensor_copy(x16, xf)
        for po in range(PO):
            nc.sync.dma_start_transpose(
                xt[:, po, tb * 128:(tb + 1) * 128], x16[:, po * 128:(po + 1) * 128]
            )
    for hp in range(HP):
        ps = mmp.tile([128, 512], F32, tag="mm")
        for po in range(PO):
            nc.tensor.matmul(
                ps, lhsT=wk_sb[:, po, hp * 128:(hp + 1) * 128], rhs=xt[:, po, :],
                start=(po == 0), stop=(po == PO - 1),
            )
        nc.vector.tensor_copy(kt_sb[:, hp, tw * 512:(tw + 1) * 512], ps)
    for hp in range(HP):
        ps = mmp.tile([128, 512], F32, tag="mm")
        for po in range(PO):
            nc.tensor.matmul(
                ps, lhsT=wq_sb[:, po, hp * 128:(hp + 1) * 128], rhs=xt[:, po, :],
                start=(po == 0), stop=(po == PO - 1),
            )
        nc.scalar.activation(qt_sb[:, hp, tw * 512:(tw + 1) * 512], ps, AF.Copy,
                             scale=SCALE)
    for tb in range(4):
        kb = tw * 4 + tb
        ps = mmp.tile([128, 512], F32, tag="mm")
        for po in range(PO):
            nc.tensor.matmul(
                ps, lhsT=xt[:, po, tb * 128:(tb + 1) * 128], rhs=wv_sb[:, po, :],
                start=(po == 0), stop=(po == PO - 1),
            )
        nc.vector.tensor_copy(
            v_sb[:, kb, :, 0:64], ps.rearrange("p (h d) -> p h d", h=HL)
        )


def build_nc():
    nc = bacc.Bacc("TRN2", target_bir_lowering=False)
    x_d = nc.dram_tensor("x", [T, D], F32, kind="ExternalInput")
    wq_d = nc.dram_tensor("wq", [D, CD], F32, kind="ExternalInput")
    wk_d = nc.dram_tensor("wk", [D, CD], F32, kind="ExternalInput")
    wv_d = nc.dram_tensor("wv", [D, CD], F32, kind="ExternalInput")
    wo_d = nc.dram_tensor("wo", [CD, D], F32, kind="ExternalInput")
    out_d = nc.dram_tensor("out", [T, D], F32, kind="ExternalOutput")

    with tile.TileContext(nc) as tc:
        with (
            tc.tile_pool(name="consts", bufs=1) as consts,
            tc.tile_pool(name="wsb", bufs=1) as wsb,
            tc.tile_pool(name="wstage", bufs=2) as wstage,
            tc.tile_pool(name="xstage", bufs=3) as xstage,
            tc.tile_pool(name="x16", bufs=3) as x16p,
            tc.tile_pool(name="xt", bufs=2) as xtp,
            tc.tile_pool(name="big", bufs=1) as big,
            tc.tile_pool(name="pt", bufs=4) as ptp,
            tc.tile_pool(name="ctxsb", bufs=2) as ctxsbp,
            tc.tile_pool(name="ctxt", bufs=2) as ctxtp,
            tc.tile_pool(name="small", bufs=4) as smallp,
            tc.tile_pool(name="osb", bufs=2) as osbp,
            tc.tile_pool(name="mm", bufs=2, space="PSUM") as mmp,
            tc.tile_pool(name="st", bufs=2, space="PSUM") as stp,
            tc.tile_pool(name="cx", bufs=2, space="PSUM") as cxp,
        ):
            ident = consts.tile([128, 128], BF16, tag="ident")
            make_identity(nc, ident)
            utri = consts.tile([128, 128], BF16, tag="utri")
            make_lower_triangular(nc, utri, val=NEG, diag=False)

            wq_sb = wsb.tile([128, PO, CD], BF16, tag="wq")
            wk_sb = wsb.tile([128, PO, CD], BF16, tag="wk")
            wv_sb = wsb.tile([128, PO, CD], BF16, tag="wv")
            wo_sb = wsb.tile([128, FC, D], BF16, tag="wo")
            for dram, sb, shp in (
                (wq_d, wq_sb, (PO, CD)),
                (wk_d, wk_sb, (PO, CD)),
                (wv_d, wv_sb, (PO, CD)),
                (wo_d, wo_sb, (FC, D)),
            ):
                stg = wstage.tile([128, shp[0], shp[1]], F32, tag="ws")
                nc.scalar.dma_start(stg, dram.rearrange("(po p) n -> p po n", p=128))
                nc.gpsimd.tensor_copy(sb, stg)

            kt_sb = big.tile([128, HP, T], BF16, tag="kt")
            qt_sb = big.tile([128, HP, T], BF16, tag="qt")
            v_sb = big.tile([128, NB, HL, 65], BF16, tag="v")
            nc.gpsimd.memset(v_sb[:, :, :, 64:65], 1.0)

            for half in range(2):
                for tw in range(2 * half, 2 * half + 2):
                    _emit_proj_tw(nc, tw, x_d, xstage, x16p, xtp, mmp,
                                  wq_sb, wk_sb, wv_sb, kt_sb, qt_sb, v_sb)
                for qb in range(8 * half, 8 * half + 8):
                    ctx_sb = ctxsbp.tile([128, HL, 64], BF16, tag="ctx")
                    _emit_attention_qb(nc, qb, kt_sb, qt_sb, v_sb, ident, utri,
                                       ptp, stp, cxp, smallp, ctx_sb)
                    ctxt_sb = ctxtp.tile([128, FC, 128], BF16, tag="ctxt")
                    _emit_out_proj_qb(nc, qb, ctx_sb, ctxt_sb, wo_sb, ident,
                                      mmp, osbp, out_d)

    nc.compile()
    return nc


_CACHE = {}


def _get_nc():
    if "nc" not in _CACHE:
        _CACHE["nc"] = build_nc()
    return _CACHE["nc"]


def make_in_maps(x, Wq, Wk, Wv, Wo):
    x = np.asarray(x, np.float32)
    Wq = np.asarray(Wq, np.float32)
    Wk = np.asarray(Wk, np.float32)
    Wv = np.asarray(Wv, np.float32)
    Wo = np.asarray(Wo, np.float32)
    in_maps = []
    for c in range(8):
        b, hh = c // 2, c % 2
        cols = slice(hh * CD, (hh + 1) * CD)
        in_maps.append({
            "x": np.ascontiguousarray(x[b]),
            "wq": np.ascontiguousarray(Wq[:, cols]),
            "wk": np.ascontiguousarray(Wk[:, cols]),
            "wv": np.ascontiguousarray(Wv[:, cols]),
            "wo": np.ascontiguousarray(Wo[cols, :]),
        })
    return in_maps


def gather_output(results, bo):
    bo = np.asarray(bo, np.float32)
    out = np.empty((B, T, D), np.float32)
    for b in range(B):
        out[b] = results[2 * b]["out"] + results[2 * b + 1]["out"] + bo[None, :]
    return out


def kernel(x, Wq, Wk, Wv, Wo, bo):
    nc = _get_nc()
    in_maps = make_in_maps(x, Wq, Wk, Wv, Wo)
    res = run_bass_kernel_spmd(nc, in_maps, core_ids=list(range(8)))
    return gather_output(res.results, bo)


# revision 9
# speedup vs baseline: 1.3436x; 1.3436x over previous
"""Multi-head causal self-attention (B=4, T=2048, D=1024, H=16) on 8 TRN2 cores.

Sharding (hardcoded): data-parallel over the 4 batches x tensor-parallel over
head halves. Core c handles batch c//2 and local heads (c%2)*8 .. (c%2)*8+7
for all 2048 positions. Every core runs the same SPMD program on its slice:

  x[b] [2048,1024] -> bf16 -> DRAM -> 8 wide DMA transposes -> x^T in SBUF
  Q^T = (Wq_slice)^T x^T / 8,  K^T = (Wk_slice)^T x^T       [dh-pairs packed
  V   = x Wv_slice (+ ones column for the softmax denominator)   on 128 parts]
  S^T = K Q^T per 128x256 block (two query blocks share one matmul; causal
        blocks only; head pairs run concurrently via PE row groups),
  P^T = exp(S^T) (ScalarE; diagonal blocks get -1e9 via a triangular matmul),
  ctx = P V  (the 65th V column accumulates the softmax denominator l),
  ctx /= l, transpose via PE, partial_out = ctx^T.T @ Wo_slice.

The host sums the two partial outputs per batch and adds the bias bo.
"""
import numpy as np

import concourse.bass as bass
import concourse.mybir as mybir
import concourse.tile as tile
from concourse import bacc
from concourse.bass_utils import run_bass_kernel_spmd
from concourse.masks import make_identity, make_lower_triangular

F32 = mybir.dt.float32
BF16 = mybir.dt.bfloat16
AF = mybir.ActivationFunctionType

B, T, D = 4, 2048, 1024
HL = 8              # local heads per core
HP = HL // 2        # local head pairs (two heads share 128 partitions)
DH = 64
PO = D // 128       # contraction chunks over D
CD = HL * DH        # 512: local context feature dim
FC = CD // 128      # 4
NB = T // 128       # 16 query/key blocks of 128
NTW = T // 512      # 4 proj column groups
SCALE = 1.0 / 8.0   # 1/sqrt(DH)
NEG = -1e9
CHUNK = 4           # key blocks per S^T psum tile ([128, 4*256] = 2 banks)


def _emit_attention_pair(nc, qb0, kt_sb, qt_sb, v_sb, ident, utri,
                         ptp, stp, cxp, smallp, ctx_sbs):
    """Attention for query blocks (qb0, qb0+1), all 4 local head pairs.

    S^T blocks for the two query blocks are computed in one N=256 matmul per
    key block (kb <= qb0); kb == qb0+1 only feeds the second query block.
    """
    qb1 = qb0 + 1
    nkb = qb1 + 1          # key blocks needed in total (for qb1)
    for hp in range(HP):
        pt_e = ptp.tile([128, NB, 256], BF16, tag="pt")
        pt_o = ptp.tile([128, NB, 256], BF16, tag="pt")
        nchunks = (nkb + CHUNK - 1) // CHUNK
        for ch in range(nchunks):
            k0 = ch * CHUNK
            k1 = min(nkb, k0 + CHUNK)
            st_e = stp.tile([128, 256 * CHUNK], F32, tag="st")
            st_o = stp.tile([128, 256 * CHUNK], F32, tag="st")
            for kb in range(k0, k1):
                w = (kb - k0) * 256
                for st, lo in ((st_e, 0), (st_o, 64)):
                    if kb <= qb0:
                        # both query blocks in one matmul
                        nc.tensor.matmul(
                            st[:, w:w + 256],
                            lhsT=kt_sb[lo:lo + 64, hp, kb * 128:(kb + 1) * 128],
                            rhs=qt_sb[lo:lo + 64, hp, qb0 * 128:qb0 * 128 + 256],
                            start=True, stop=kb != qb0,
                        )
                        if kb == qb0:   # diagonal for qb0: mask cols 0..127
                            nc.tensor.matmul(
                                st[:, w:w + 128],
                                lhsT=ident, rhs=utri, start=False, stop=True,
                            )
                    else:  # kb == qb1: second query block only (diagonal)
                        nc.tensor.matmul(
                            st[:, w + 128:w + 256],
                            lhsT=kt_sb[lo:lo + 64, hp, kb * 128:(kb + 1) * 128],
                            rhs=qt_sb[lo:lo + 64, hp, qb1 * 128:(qb1 + 1) * 128],
                            start=True, stop=False,
                        )
                        nc.tensor.matmul(
                            st[:, w + 128:w + 256],
                            lhsT=ident, rhs=utri, start=False, stop=True,
                        )
            ncol = (k1 - k0) * 256
            nc.scalar.activation(pt_e[:, k0:k1, :], st_e[:, :ncol], AF.Exp)
            nc.scalar.activation(pt_o[:, k0:k1, :], st_o[:, :ncol], AF.Exp)
        # AV per query block, sequentially: the two chains must not share a
        # PSUM bank and only 2 cx banks are budgeted per head pair
        for q in (0, 1):
            qb = qb0 + q
            ctx_e = cxp.tile([128, 65], F32, tag="cx")
            ctx_o = cxp.tile([128, 65], F32, tag="cx")
            for kb in range(qb + 1):
                for par, ctx_ps, pt in ((0, ctx_e, pt_e), (1, ctx_o, pt_o)):
                    nc.tensor.matmul(
                        ctx_ps,
                        lhsT=pt[:, kb, q * 128:(q + 1) * 128],
                        rhs=v_sb[:, kb, 2 * hp + par, :],
                        start=(kb == 0), stop=(kb == qb),
                    )
            ctmp = smallp.tile([128, 130], F32, tag="ctmp")
            nc.vector.tensor_copy(ctmp[:, 0:65], ctx_e)
            nc.vector.tensor_copy(ctmp[:, 65:130], ctx_o)
            linv = smallp.tile([128, 2], F32, tag="linv")
            nc.vector.reciprocal(linv, ctmp[:, 64::65])
            nc.vector.tensor_scalar_mul(
                ctx_sbs[q][:, 2 * hp, :], ctmp[:, 0:64], linv[:, 0:1])
            nc.vector.tensor_scalar_mul(
                ctx_sbs[q][:, 2 * hp + 1, :], ctmp[:, 65:129], linv[:, 1:2])


def _emit_out_proj_qb(nc, qb, ctx_sb, ctxt_sb, wo_sb, ident, mmp, osbp, out_d):
    """ctx^T via PE transpose, then the output projection for one q block."""
    for fc in range(FC):
        tp = mmp.tile([128, 128], BF16, tag="mm")
        nc.tensor.transpose(tp, ctx_sb[:, 2 * fc:2 * fc + 2, :], ident)
        nc.vector.tensor_copy(ctxt_sb[:, fc, :], tp)
    for dw in range(2):
        ps = mmp.tile([128, 512], F32, tag="mm")
        for fc in range(FC):
            nc.tensor.matmul(
                ps, lhsT=ctxt_sb[:, fc, :], rhs=wo_sb[:, fc, dw * 512:(dw + 1) * 512],
                start=(fc == 0), stop=(fc == FC - 1),
            )
        osb = osbp.tile([128, 512], F32, tag="osb")
        nc.vector.tensor_copy(osb, ps)
        nc.scalar.dma_start(out_d[qb * 128:(qb + 1) * 128, dw * 512:(dw + 1) * 512], osb)


def _emit_proj_tw(nc, tw, xt_sb, mmp, wq_sb, wk_sb, wv_sb, kt_sb, qt_sb, v_sb):
    """Project K^T, Q^T, V for one 512-column group of x^T."""
    tsl = slice(tw * 512, (tw + 1) * 512)
    for hp in range(HP):
        ps = mmp.tile([128, 512], F32, tag="mm")
        for po in range(PO):
            nc.tensor.matmul(
                ps, lhsT=wk_sb[:, po, hp * 128:(hp + 1) * 128], rhs=xt_sb[:, po, tsl],
                start=(po == 0), stop=(po == PO - 1),
            )
        nc.vector.tensor_copy(kt_sb[:, hp, tsl], ps)
    for hp in range(HP):
        ps = mmp.tile([128, 512], F32, tag="mm")
        for po in range(PO):
            nc.tensor.matmul(
                ps, lhsT=wq_sb[:, po, hp * 128:(hp + 1) * 128], rhs=xt_sb[:, po, tsl],
                start=(po == 0), stop=(po == PO - 1),
            )
        nc.scalar.activation(qt_sb[:, hp, tsl], ps, AF.Copy, scale=SCALE)
    for tb in range(4):
        kb = tw * 4 + tb
        ps = mmp.tile([128, 512], F32, tag="mm")
        for po in range(PO):
            nc.tensor.matmul(
                ps, lhsT=xt_sb[:, po, kb * 128:(kb + 1) * 128], rhs=wv_sb[:, po, :],
                start=(po == 0), stop=(po == PO - 1),
            )
        nc.vector.tensor_copy(
            v_sb[:, kb, :, 0:64], ps.rearrange("p (h d) -> p h d", h=HL)
        )


def build_nc():
    nc = bacc.Bacc("TRN2", target_bir_lowering=False)
    x_d = nc.dram_tensor("x", [T, D], F32, kind="ExternalInput")
    wq_d = nc.dram_tensor("wq", [D, CD], F32, kind="ExternalInput")
    wk_d = nc.dram_tensor("wk", [D, CD], F32, kind="ExternalInput")
    wv_d = nc.dram_tensor("wv", [D, CD], F32, kind="ExternalInput")
    wo_d = nc.dram_tensor("wo", [CD, D], F32, kind="ExternalInput")
    out_d = nc.dram_tensor("out", [T, D], F32, kind="ExternalOutput")
    xb16_d = nc.dram_tensor("xb16", [T, D], BF16)  # internal scratch

    with tile.TileContext(nc) as tc:
        with (
            tc.tile_pool(name="consts", bufs=1) as consts,
            tc.tile_pool(name="wsb", bufs=1) as wsb,
            tc.tile_pool(name="wstage", bufs=1) as wstage,
            tc.tile_pool(name="xstage", bufs=3) as xstage,
            tc.tile_pool(name="x16", bufs=3) as x16p,
            tc.tile_pool(name="big", bufs=1) as big,
            tc.tile_pool(name="pt", bufs=3) as ptp,
            tc.tile_pool(name="ctxsb", bufs=4) as ctxsbp,
            tc.tile_pool(name="ctxt", bufs=2) as ctxtp,
            tc.tile_pool(name="small", bufs=4) as smallp,
            tc.tile_pool(name="osb", bufs=2) as osbp,
            tc.tile_pool(name="mm", bufs=2, space="PSUM") as mmp,
            tc.tile_pool(name="st", bufs=2, space="PSUM") as stp,
            tc.tile_pool(name="cx", bufs=2, space="PSUM") as cxp,
        ):
            ident = consts.tile([128, 128], BF16, tag="ident")
            make_identity(nc, ident)
            utri = consts.tile([128, 128], BF16, tag="utri")
            make_lower_triangular(nc, utri, val=NEG, diag=False)

            wq_sb = wsb.tile([128, PO, CD], BF16, tag="wq")
            wk_sb = wsb.tile([128, PO, CD], BF16, tag="wk")
            wv_sb = wsb.tile([128, PO, CD], BF16, tag="wv")
            wo_sb = wsb.tile([128, FC, D], BF16, tag="wo")
            for i, (dram, sb, shp) in enumerate((
                (wq_d, wq_sb, (PO, CD)),
                (wk_d, wk_sb, (PO, CD)),
                (wv_d, wv_sb, (PO, CD)),
                (wo_d, wo_sb, (FC, D)),
            )):
                stg = wstage.tile([128, shp[0], shp[1]], F32, tag="ws")
                nc.scalar.dma_start(stg, dram.rearrange("(po p) n -> p po n", p=128))
                if i % 2 == 0:
                    nc.vector.tensor_copy(sb, stg)
                else:
                    nc.scalar.activation(sb, stg, AF.Copy)

            # x -> bf16 in DRAM, then 8 wide DMA transposes -> x^T in SBUF
            for tb in range(NB):
                xf = xstage.tile([128, D], F32, tag="xf")
                nc.scalar.dma_start(xf, x_d[tb * 128:(tb + 1) * 128, :])
                x16 = x16p.tile([128, D], BF16, tag="x16")
                if tb % 2 == 0:
                    nc.vector.tensor_copy(x16, xf)
                else:
                    nc.scalar.activation(x16, xf, AF.Copy)
                nc.sync.dma_start(xb16_d[tb * 128:(tb + 1) * 128, :], x16)

            xt_sb = big.tile([128, PO, T], BF16, tag="xt")
            for po in range(PO):
                nc.sync.dma_start_transpose(
                    xt_sb[:, po, :], xb16_d[:, po * 128:(po + 1) * 128])

            kt_sb = big.tile([128, HP, T], BF16, tag="kt")
            qt_sb = big.tile([128, HP, T], BF16, tag="qt")
            v_sb = big.tile([128, NB, HL, 65], BF16, tag="v")
            nc.gpsimd.memset(v_sb[:, :, :, 64:65], 1.0)

            for half in range(2):
                for tw in range(2 * half, 2 * half + 2):
                    _emit_proj_tw(nc, tw, xt_sb, mmp,
                                  wq_sb, wk_sb, wv_sb, kt_sb, qt_sb, v_sb)
                for qb0 in range(8 * half, 8 * half + 8, 2):
                    ctx_sbs = [ctxsbp.tile([128, HL, 64], BF16, tag="ctx",
                                           name=f"ctxsb{q}")
                               for q in range(2)]
                    _emit_attention_pair(nc, qb0, kt_sb, qt_sb, v_sb, ident,
                                         utri, ptp, stp, cxp, smallp, ctx_sbs)
                    for q in (0, 1):
                        ctxt_sb = ctxtp.tile([128, FC, 128], BF16, tag="ctxt")
                        _emit_out_proj_qb(nc, qb0 + q, ctx_sbs[q], ctxt_sb,
                                          wo_sb, ident, mmp, osbp, out_d)

    nc.compile()
    return nc


_CACHE = {}


def _get_nc():
    if "nc" not in _CACHE:
        _CACHE["nc"] = build_nc()
    return _CACHE["nc"]


def make_in_maps(x, Wq, Wk, Wv, Wo):
    x = np.asarray(x, np.float32)
    Wq = np.asarray(Wq, np.float32)
    Wk = np.asarray(Wk, np.float32)
    Wv = np.asarray(Wv, np.float32)
    Wo = np.asarray(Wo, np.float32)
    in_maps = []
    for c in range(8):
        b, hh = c // 2, c % 2
        cols = slice(hh * CD, (hh + 1) * CD)
        in_maps.append({
            "x": np.ascontiguousarray(x[b]),
            "wq": np.ascontiguousarray(Wq[:, cols]),
            "wk": np.ascontiguousarray(Wk[:, cols]),
            "wv": np.ascontiguousarray(Wv[:, cols]),
            "wo": np.ascontiguousarray(Wo[cols, :]),
        })
    return in_maps


def gather_output(results, bo):
    bo = np.asarray(bo, np.float32)
    out = np.empty((B, T, D), np.float32)
    for b in range(B):
        out[b] = results[2 * b]["out"] + results[2 * b + 1]["out"] + bo[None, :]
    return out


def kernel(x, Wq, Wk, Wv, Wo, bo):
    nc = _get_nc()
    in_maps = make_in_maps(x, Wq, Wk, Wv, Wo)
    res = run_bass_kernel_spmd(nc, in_maps, core_ids=list(range(8)))
    return gather_output(res.results, bo)


# revision 16
# speedup vs baseline: 1.3839x; 1.0300x over previous
"""Multi-head causal self-attention (B=4, T=2048, D=1024, H=16) on 8 TRN2 cores.

Sharding (hardcoded): data-parallel over the 4 batches x tensor-parallel over
head halves. Core c handles batch c//2 and local heads (c%2)*8 .. (c%2)*8+7
for all 2048 positions. Every core runs the same SPMD program on its slice:

  x[b] [2048,1024] -> bf16 -> DRAM -> 8 wide DMA transposes -> x^T in SBUF
  Q^T = (Wq_slice)^T x^T / 8,  K^T = (Wk_slice)^T x^T       [dh-pairs packed
  V   = x Wv_slice (+ ones column for the softmax denominator)   on 128 parts]
  S^T = K Q^T per 128x256 block (two query blocks share one matmul; causal
        blocks only; head pairs run concurrently via PE row groups),
  P^T = exp(S^T) (ScalarE; diagonal blocks get -1e9 via a triangular matmul),
  ctx = P V  (the 65th V column accumulates the softmax denominator l),
  ctx /= l, transpose via PE, partial_out = ctx^T.T @ Wo_slice.

The host sums the two partial outputs per batch and adds the bias bo.
"""
import numpy as np

import concourse.bass as bass
import concourse.mybir as mybir
import concourse.tile as tile
from concourse import bacc
from concourse.bass_utils import run_bass_kernel_spmd
from concourse.masks import make_identity, make_lower_triangular

F32 = mybir.dt.float32
BF16 = mybir.dt.bfloat16
AF = mybir.ActivationFunctionType

B, T, D = 4, 2048, 1024
HL = 8              # local heads per core
HP = HL // 2        # local head pairs (two heads share 128 partitions)
DH = 64
PO = D // 128       # contraction chunks over D
CD = HL * DH        # 512: local context feature dim
FC = CD // 128      # 4
NB = T // 128       # 16 query/key blocks of 128
NTW = T // 512      # 4 proj column groups
SCALE = 1.0 / 8.0   # 1/sqrt(DH)
NEG = -1e9
CHUNK = 4           # key blocks per S^T psum tile ([128, 4*256] = 2 banks)


def _emit_attention_pair(nc, qb0, kt_sb, qt_sb, v_sb, ident, utri,
                         ptp, stp, cxp, smallp, ctx_sbs):
    """Attention for query blocks (qb0, qb0+1), all 4 local head pairs.

    S^T blocks for the two query blocks are computed in one N=256 matmul per
    key block (kb <= qb0); kb == qb0+1 only feeds the second query block.
    """
    qb1 = qb0 + 1
    nkb = qb1 + 1          # key blocks needed in total (for qb1)
    for hp in range(HP):
        pt_e = ptp.tile([128, NB, 256], BF16, tag="pt")
        pt_o = ptp.tile([128, NB, 256], BF16, tag="pt")
        nchunks = (nkb + CHUNK - 1) // CHUNK
        for ch in range(nchunks):
            k0 = ch * CHUNK
            k1 = min(nkb, k0 + CHUNK)
            st_e = stp.tile([128, 256 * CHUNK], F32, tag="st")
            st_o = stp.tile([128, 256 * CHUNK], F32, tag="st")
            for kb in range(k0, k1):
                w = (kb - k0) * 256
                for st, lo in ((st_e, 0), (st_o, 64)):
                    if kb <= qb0:
                        # both query blocks in one matmul
                        nc.tensor.matmul(
                            st[:, w:w + 256],
                            lhsT=kt_sb[lo:lo + 64, hp, kb * 128:(kb + 1) * 128],
                            rhs=qt_sb[lo:lo + 64, hp, qb0 * 128:qb0 * 128 + 256],
                            start=True, stop=kb != qb0,
                        )
                        if kb == qb0:   # diagonal for qb0: mask cols 0..127
                            nc.tensor.matmul(
                                st[:, w:w + 128],
                                lhsT=ident, rhs=utri, start=False, stop=True,
                            )
                    else:  # kb == qb1: second query block only (diagonal)
                        nc.tensor.matmul(
                            st[:, w + 128:w + 256],
                            lhsT=kt_sb[lo:lo + 64, hp, kb * 128:(kb + 1) * 128],
                            rhs=qt_sb[lo:lo + 64, hp, qb1 * 128:(qb1 + 1) * 128],
                            start=True, stop=False,
                        )
                        nc.tensor.matmul(
                            st[:, w + 128:w + 256],
                            lhsT=ident, rhs=utri, start=False, stop=True,
                        )
            ncol = (k1 - k0) * 256
            nc.scalar.activation(pt_e[:, k0:k1, :], st_e[:, :ncol], AF.Exp)
            nc.scalar.activation(pt_o[:, k0:k1, :], st_o[:, :ncol], AF.Exp)
        # AV per query block, sequentially: the two chains must not share a
        # PSUM bank and only 2 cx banks are budgeted per head pair
        for q in (0, 1):
            qb = qb0 + q
            ctx_e = cxp.tile([128, 65], F32, tag="cx")
            ctx_o = cxp.tile([128, 65], F32, tag="cx")
            for kb in range(qb + 1):
                for par, ctx_ps, pt in ((0, ctx_e, pt_e), (1, ctx_o, pt_o)):
                    nc.tensor.matmul(
                        ctx_ps,
                        lhsT=pt[:, kb, q * 128:(q + 1) * 128],
                        rhs=v_sb[:, kb, 2 * hp + par, :],
                        start=(kb == 0), stop=(kb == qb),
                    )
            ctmp = smallp.tile([128, 130], F32, tag="ctmp")
            nc.vector.tensor_copy(ctmp[:, 0:65], ctx_e)
            nc.vector.tensor_copy(ctmp[:, 65:130], ctx_o)
            linv = smallp.tile([128, 2], F32, tag="linv")
            nc.vector.reciprocal(linv, ctmp[:, 64::65])
            nc.vector.tensor_scalar_mul(
                ctx_sbs[q][:, 2 * hp, :], ctmp[:, 0:64], linv[:, 0:1])
            nc.vector.tensor_scalar_mul(
                ctx_sbs[q][:, 2 * hp + 1, :], ctmp[:, 65:129], linv[:, 1:2])


def _emit_out_proj_qb(nc, qb, ctx_sb, ctxt_sb, wo_sb, ident, mmp, osbp, out_d):
    """ctx^T via PE transpose, then the output projection for one q block."""
    for fc in range(FC):
        tp = mmp.tile([128, 128], BF16, tag="mm")
        nc.tensor.transpose(tp, ctx_sb[:, 2 * fc:2 * fc + 2, :], ident)
        nc.vector.tensor_copy(ctxt_sb[:, fc, :], tp)
    for dw in range(2):
        ps = mmp.tile([128, 512], F32, tag="mm")
        for fc in range(FC):
            nc.tensor.matmul(
                ps, lhsT=ctxt_sb[:, fc, :], rhs=wo_sb[:, fc, dw * 512:(dw + 1) * 512],
                start=(fc == 0), stop=(fc == FC - 1),
            )
        osb = osbp.tile([128, 512], F32, tag="osb")
        nc.vector.tensor_copy(osb, ps)
        nc.scalar.dma_start(out_d[qb * 128:(qb + 1) * 128, dw * 512:(dw + 1) * 512], osb)


def _emit_proj_tw(nc, tw, x_d, identf, xstage, xt_sb, mmp,
                  wq_sb, wk_sb, wv_sb, kt_sb, qt_sb, v_sb):
    """x^T for one 512-column group via PE transpose (load f32 rows,
    transpose each 128x128 block on the TensorEngine, cast to bf16 on the
    PSUM->SBUF copy), then project K^T, Q^T, V for it."""
    for tb4 in range(4):
        tb = tw * 4 + tb4
        xf = xstage.tile([128, D], F32, tag="xf")
        nc.scalar.dma_start(xf, x_d[tb * 128:(tb + 1) * 128, :])
        for po in range(PO):
            tp = mmp.tile([128, 128], F32, tag="mm", name="xtp")
            nc.tensor.transpose(tp, xf[:, po * 128:(po + 1) * 128], identf)
            if po % 2 == 0:
                nc.vector.tensor_copy(xt_sb[:, po, tb * 128:(tb + 1) * 128], tp)
            else:
                nc.scalar.activation(xt_sb[:, po, tb * 128:(tb + 1) * 128], tp,
                                     AF.Copy)
    tsl = slice(tw * 512, (tw + 1) * 512)
    for hp in range(HP):
        ps = mmp.tile([128, 512], F32, tag="mm")
        for po in range(PO):
            nc.tensor.matmul(
                ps, lhsT=wk_sb[:, po, hp * 128:(hp + 1) * 128], rhs=xt_sb[:, po, tsl],
                start=(po == 0), stop=(po == PO - 1),
            )
        nc.vector.tensor_copy(kt_sb[:, hp, tsl], ps)
    for hp in range(HP):
        ps = mmp.tile([128, 512], F32, tag="mm")
        for po in range(PO):
            nc.tensor.matmul(
                ps, lhsT=wq_sb[:, po, hp * 128:(hp + 1) * 128], rhs=xt_sb[:, po, tsl],
                start=(po == 0), stop=(po == PO - 1),
            )
        nc.scalar.activation(qt_sb[:, hp, tsl], ps, AF.Copy, scale=SCALE)
    for tb in range(4):
        kb = tw * 4 + tb
        ps = mmp.tile([128, 512], F32, tag="mm")
        for po in range(PO):
            nc.tensor.matmul(
                ps, lhsT=xt_sb[:, po, kb * 128:(kb + 1) * 128], rhs=wv_sb[:, po, :],
                start=(po == 0), stop=(po == PO - 1),
            )
        nc.vector.tensor_copy(
            v_sb[:, kb, :, 0:64], ps.rearrange("p (h d) -> p h d", h=HL)
        )


def build_nc():
    nc = bacc.Bacc("TRN2", target_bir_lowering=False)
    x_d = nc.dram_tensor("x", [T, D], F32, kind="ExternalInput")
    wq_d = nc.dram_tensor("wq", [D, CD], F32, kind="ExternalInput")
    wk_d = nc.dram_tensor("wk", [D, CD], F32, kind="ExternalInput")
    wv_d = nc.dram_tensor("wv", [D, CD], F32, kind="ExternalInput")
    wo_d = nc.dram_tensor("wo", [CD, D], F32, kind="ExternalInput")
    out_d = nc.dram_tensor("out", [T, D], F32, kind="ExternalOutput")

    with tile.TileContext(nc) as tc:
        with (
            tc.tile_pool(name="consts", bufs=1) as consts,
            tc.tile_pool(name="wsb", bufs=1) as wsb,
            tc.tile_pool(name="wstage", bufs=1) as wstage,
            tc.tile_pool(name="xstage", bufs=3) as xstage,
            tc.tile_pool(name="big", bufs=1) as big,
            tc.tile_pool(name="pt", bufs=3) as ptp,
            tc.tile_pool(name="ctxsb", bufs=4) as ctxsbp,
            tc.tile_pool(name="ctxt", bufs=2) as ctxtp,
            tc.tile_pool(name="small", bufs=4) as smallp,
            tc.tile_pool(name="osb", bufs=2) as osbp,
            tc.tile_pool(name="mm", bufs=2, space="PSUM") as mmp,
            tc.tile_pool(name="st", bufs=2, space="PSUM") as stp,
            tc.tile_pool(name="cx", bufs=2, space="PSUM") as cxp,
        ):
            ident = consts.tile([128, 128], BF16, tag="ident")
            make_identity(nc, ident)
            identf = consts.tile([128, 128], F32, tag="identf")
            make_identity(nc, identf)
            utri = consts.tile([128, 128], BF16, tag="utri")
            make_lower_triangular(nc, utri, val=NEG, diag=False)

            wq_sb = wsb.tile([128, PO, CD], BF16, tag="wq")
            wk_sb = wsb.tile([128, PO, CD], BF16, tag="wk")
            wv_sb = wsb.tile([128, PO, CD], BF16, tag="wv")
            wo_sb = wsb.tile([128, FC, D], BF16, tag="wo")
            for i, (dram, sb, shp) in enumerate((
                (wq_d, wq_sb, (PO, CD)),
                (wk_d, wk_sb, (PO, CD)),
                (wv_d, wv_sb, (PO, CD)),
                (wo_d, wo_sb, (FC, D)),
            )):
                stg = wstage.tile([128, shp[0], shp[1]], F32, tag="ws")
                nc.scalar.dma_start(stg, dram.rearrange("(po p) n -> p po n", p=128))
                if i % 2 == 0:
                    nc.vector.tensor_copy(sb, stg)
                else:
                    nc.scalar.activation(sb, stg, AF.Copy)

            xt_sb = big.tile([128, PO, T], BF16, tag="xt")
            kt_sb = big.tile([128, HP, T], BF16, tag="kt")
            qt_sb = big.tile([128, HP, T], BF16, tag="qt")
            v_sb = big.tile([128, NB, HL, 65], BF16, tag="v")
            nc.gpsimd.memset(v_sb[:, :, :, 64:65], 1.0)

            for half in range(2):
                for tw in range(2 * half, 2 * half + 2):
                    _emit_proj_tw(nc, tw, x_d, identf, xstage, xt_sb, mmp,
                                  wq_sb, wk_sb, wv_sb, kt_sb, qt_sb, v_sb)
                for qb0 in range(8 * half, 8 * half + 8, 2):
                    ctx_sbs = [ctxsbp.tile([128, HL, 64], BF16, tag="ctx",
                                           name=f"ctxsb{q}")
                               for q in range(2)]
                    _emit_attention_pair(nc, qb0, kt_sb, qt_sb, v_sb, ident,
                                         utri, ptp, stp, cxp, smallp, ctx_sbs)
                    for q in (0, 1):
                        ctxt_sb = ctxtp.tile([128, FC, 128], BF16, tag="ctxt")
                        _emit_out_proj_qb(nc, qb0 + q, ctx_sbs[q], ctxt_sb,
                                          wo_sb, ident, mmp, osbp, out_d)

    nc.compile()
    return nc


_CACHE = {}


def _get_nc():
    if "nc" not in _CACHE:
        _CACHE["nc"] = build_nc()
    return _CACHE["nc"]


def make_in_maps(x, Wq, Wk, Wv, Wo):
    x = np.asarray(x, np.float32)
    Wq = np.asarray(Wq, np.float32)
    Wk = np.asarray(Wk, np.float32)
    Wv = np.asarray(Wv, np.float32)
    Wo = np.asarray(Wo, np.float32)
    in_maps = []
    for c in range(8):
        b, hh = c // 2, c % 2
        cols = slice(hh * CD, (hh + 1) * CD)
        in_maps.append({
            "x": np.ascontiguousarray(x[b]),
            "wq": np.ascontiguousarray(Wq[:, cols]),
            "wk": np.ascontiguousarray(Wk[:, cols]),
            "wv": np.ascontiguousarray(Wv[:, cols]),
            "wo": np.ascontiguousarray(Wo[cols, :]),
        })
    return in_maps


def gather_output(results, bo):
    bo = np.asarray(bo, np.float32)
    out = np.empty((B, T, D), np.float32)
    for b in range(B):
        out[b] = results[2 * b]["out"] + results[2 * b + 1]["out"] + bo[None, :]
    return out


def kernel(x, Wq, Wk, Wv, Wo, bo):
    nc = _get_nc()
    in_maps = make_in_maps(x, Wq, Wk, Wv, Wo)
    res = run_bass_kernel_spmd(nc, in_maps, core_ids=list(range(8)))
    return gather_output(res.results, bo)
